# revision 36
# baseline (speedup 1.0000x reference)
"""Causal self-attention (B=2, T=2048, C=2048, H=16) on 8 TRN2 NeuronCores.

Sharding: tensor-parallel over heads (2 heads per core, both batches on every
core). Each core computes q/k/v projections for its 2 heads, RoPE, causal
softmax(qk^T)v, and a partial output projection against its slice of Wo's
columns. The host sums the 8 partial projections and adds the (linear) bias
terms.

v3 additions over v2:
  - x/wqk/wv ship pre-tiled in SBUF layout: every input transfer is a single
    contiguous block (4-16KB per-partition lines), roughly doubling the
    effective HBM rate during the DMA-bound cold start.
  - ~36 throwaway matmuls on a zeroed SBUF tile run during the initial DMA
    wait so the PE clock-gate (HAM) is already at 8/8 when real data lands.
  - RoPE tables are read through 0-stride broadcast APs over the head dim
    (no on-device duplication pass).
  - out(g-1) is emitted after attn(g)+proj(g+1), keeping out matmuls in
    reserve to fill the denominator->reciprocal->yT latency hole at the end
    of each attention block.
  - The softmax denominator is computed from the partial row-sum before the
    last exp finishes plus a cheap N=128 accumulate for the final strip.
  - The last block's out groups rotate through 4 PSUM slots (s,s,pj,y — the
    projection slots are dead by then) with ACT/DVE-split evacuation and
    per-half stores, so the kernel tail never idles the PE long enough to
    re-throttle the clock.

v2 layout strategy (all matmul operands fp16, PSUM accumulation fp32):
  - Host pre-casts x (transposed to xT [B, C, T]), weights, RoPE tables and
    masks to fp16: halves DMA bytes, removes the gpsimd casting-DMA path, and
    makes every matmul 1 cycle/row regardless of free-dim size.
  - The core's two heads are paired into single wide instructions wherever an
    ACT/DVE op would otherwise run per head: one [128, 2, 512] tile spans two
    PSUM banks (or 2KB of SBUF), so exp/mask/row-accumulate/copy pay their
    fixed per-instruction overhead once instead of twice.
  - PSUM budget (8 banks): "pj" proj pair (2) + "s" score/den/out pairs
    (2x2, double-buffered) + "y" attention accumulator pair (2). Separate
    tags keep next-block projections from queueing behind current-block
    output copies in the slot rotation.
  - Software-pipelined emission: attention(g) is emitted before proj(g+1)
    and out(g) last, so the static scheduler fills the exp round-trips of
    the attention inner loop with projection matmuls and covers the RoPE
    chain latency with output-projection work. The first two blocks'
    projections are interleaved per contraction chunk so the cold-start
    x/wqk stream stays under the HBM rate (the first chunk of each stream
    ships alone so matmul 0 issues after three small transfers).
  - q, k are produced transposed (qT/kT [head_dim, T]); scores are computed
    as S_T = kT_tile.T @ qT [keys, queries] so the softmax normalizer is a
    ones-matmul partition sum. y is produced transposed (yT = v_tile.T @ P_T)
    and feeds the output projection with no transposes anywhere.
  - exp via ACT (scale fused), fp16 out; causal masking is applied on the
    PE by accumulating a constant -1e4 upper-triangular matrix (maskm.T @ I)
    onto the diagonal-boundary 128-column strip of each score tile.
  - Row-sum accumulation (softmax denominator) runs on DVE in fp16 at 2x
    mode; the final partition-sum/broadcast is one ones-matmul per head.
"""

import sys

sys.path.insert(0, "/opt/trn_rl_repo")

import numpy as np

import concourse.bacc as bacc
import concourse.mybir as mybir
import concourse.tile as tile
from concourse import bass_utils
from concourse.bass import AP

B, T, C, H = 2, 2048, 2048, 16
HD = C // H  # 128
BASE = 10000.0
NC_ = 8  # cores
NH = H // NC_  # heads per core = 2
TB = 512  # T block
NTB = T // TB  # 4
CK = C // 128  # 16 contraction chunks
SCALE = 1.0 / float(np.sqrt(np.float32(HD)))

f32 = mybir.dt.float32
f16 = mybir.dt.float16
AF = mybir.ActivationFunctionType
OP = mybir.AluOpType

TRACE = False
LAST_RESULT = None

_STATE = {}


def _rope_tables():
    """cos/sin tables [HD, T] mirroring reference._rope_tables (f32 chain)."""
    try:
        import jax
        import jax.numpy as jnp

        cpu = jax.devices("cpu")[0]
        with jax.default_device(cpu):
            p = jnp.arange(HD // 2, dtype=jnp.float32)
            theta = jnp.power(BASE, -(2.0**p) / HD)
            pos = jnp.arange(1, T + 1, dtype=jnp.float32)[:, None]
            c = pos * theta
            ang = jnp.concatenate([c, c], axis=-1)  # [T, HD]
            cos = np.asarray(jnp.cos(ang)).T  # [HD, T]
            sin = np.asarray(jnp.sin(ang)).T
        return np.ascontiguousarray(cos), np.ascontiguousarray(sin)
    except Exception:
        p = np.arange(HD // 2, dtype=np.float32)
        theta = np.power(np.float32(BASE), (-(2.0**p) / HD).astype(np.float32))
        pos = np.arange(1, T + 1, dtype=np.float32)[:, None]
        c = (pos * theta).astype(np.float32)
        ang = np.concatenate([c, c], axis=-1)
        return (
            np.ascontiguousarray(np.cos(ang).T.astype(np.float32)),
            np.ascontiguousarray(np.sin(ang).T.astype(np.float32)),
        )


def _build_program():
    nc = bacc.Bacc("TRN2", target_bir_lowering=False, debug=False, num_devices=NC_)

    # x and wqk ship pre-tiled in SBUF layout so every transfer is fully
    # contiguous (4-16KB per-partition lines instead of 1KB strided lines:
    # ~2x the effective HBM rate during the cold start)
    d_xT = nc.dram_tensor("xT", (B, NTB, 4, 128, 4, TB), f16, kind="ExternalInput")
    d_wqk = nc.dram_tensor("wqk", (4, 128, 4, 2, NH * HD), f16, kind="ExternalInput")
    d_wv = nc.dram_tensor("wv", (128, CK, NH * HD), f16, kind="ExternalInput")
    d_wo = nc.dram_tensor("wo", (NH * HD, C), f16, kind="ExternalInput")
    d_bq = nc.dram_tensor("bq", (HD, NH), f32, kind="ExternalInput")
    d_bk = nc.dram_tensor("bk", (HD, NH), f32, kind="ExternalInput")
    d_cos = nc.dram_tensor("cosT", (HD, T), f16, kind="ExternalInput")
    d_sin = nc.dram_tensor("sinT", (HD, T), f16, kind="ExternalInput")
    d_maskm = nc.dram_tensor("maskm", (128, 128), f16, kind="ExternalInput")
    d_ident = nc.dram_tensor("ident", (128, 128), f16, kind="ExternalInput")
    d_ones = nc.dram_tensor("onesm", (128, 128), f16, kind="ExternalInput")
    d_out = nc.dram_tensor("out", (B, T, C), f16, kind="ExternalOutput")

    with tile.TileContext(nc) as tc:
        with (
            tc.tile_pool(name="w", bufs=1) as wp,
            tc.tile_pool(name="xp", bufs=1) as xp,
            tc.tile_pool(name="kv", bufs=1) as kvp,
            tc.tile_pool(name="work", bufs=1) as wk_,
            tc.tile_pool(name="ps", bufs=1, space="PSUM") as ps,
        ):
            # --- resident weights/constants ---
            # The first block's x tiles + wqk chunks are emitted first so they
            # win HWDGE/DMA priority; small constants ride the ACT HWDGE
            # queue; big late-use tables (cos/sin/wv/wo) are emitted in
            # first-use order behind them.
            wqk_t = wp.tile([128, CK, 2, NH * HD], f16, name="wqk_t")
            wv_t = wp.tile([128, CK, NH * HD], f16, name="wv_t")
            wo_t = wp.tile([128, NH, C], f16, name="wo_t")
            bq_t = wp.tile([128, NH], f32, name="bq_t")
            bk_t = wp.tile([128, NH], f32, name="bk_t")
            cos_t = wp.tile([128, T], f16, name="cos_t")
            sin_t = wp.tile([128, T], f16, name="sin_t")
            maskm_t = wp.tile([128, 128], f16, name="maskm_t")
            ident_t = wp.tile([128, 128], f16, name="ident_t")
            ones_t = wp.tile([128, 128], f16, name="ones_t")

            def load_constants():
                nc.sync.dma_start(bq_t[:], d_bq.ap()[:])
                nc.sync.dma_start(bk_t[:], d_bk.ap()[:])
                nc.sync.dma_start(maskm_t[:], d_maskm.ap()[:])
                nc.sync.dma_start(ident_t[:], d_ident.ap()[:])
                # big tables split in halves and ordered by first use so no
                # single transfer starves the first block's x/wqk stream
                nc.sync.dma_start(cos_t[:], d_cos.ap()[:])
                nc.sync.dma_start(sin_t[:], d_sin.ap()[:])
                nc.sync.dma_start(wv_t[:], d_wv.ap()[:])
                nc.sync.dma_start(ones_t[:], d_ones.ap()[:])
                for h in range(NH):
                    nc.sync.dma_start(
                        wo_t[:, h, :], d_wo.ap()[h * 128 : (h + 1) * 128, :]
                    )

            kv_state = {}

            def bc2(s):
                """[128, F] AP -> [128, NH, F] with a 0-stride head dim, so
                both heads' RoPE multiplies read the same table row."""
                return AP(s.tensor, s.offset, [list(s.ap[0]), [0, NH], list(s.ap[1])])

            def emit_x1(b, tb, j):
                """One x tile [128, 4, 512] (4 contraction chunks)."""
                xt = xp.tile([128, 4, TB], f16, tag="xt", bufs=12, name=f"x{b}{tb}{j}")
                nc.sync.dma_start(xt[:], d_xT.ap()[b, tb, j])
                return xt

            def emit_x(b, tb):
                """Stream one x block: 4 DMAs of [128, 4, 512]."""
                return [emit_x1(b, tb, j) for j in range(4)]

            def alloc_kv(b):
                ktab = kvp.tile([128, NH, T], f16, tag="kt", bufs=2, name=f"ktab_{b}")
                vt = kvp.tile(
                    [128, T // 128, NH * HD], f16, tag="v", bufs=2, name=f"vt_{b}"
                )
                kv_state[b] = (ktab, vt)

            def emit_rope(b, tb, prj, b_t, is_q, qT):
                """PSUM projection pair -> fp16 qT / ktab slice with RoPE.

                The two PSUM->SBUF bias copies are split across ACT and DVE so
                the projection PSUM slot frees after ~0.7us, not ~1.3us."""
                ktab, _ = kv_state[b]
                tbs = slice(tb * TB, (tb + 1) * TB)
                qb = wk_.tile([128, NH, TB], f16, tag="qb", bufs=2)
                nc.scalar.activation(
                    qb[:, 0, :], prj[:, 0, :], AF.Identity, bias=b_t[:, 0:1]
                )
                nc.vector.tensor_scalar_add(qb[:, 1, :], prj[:, 1, :], b_t[:, 1:2])
                rotb = wk_.tile([128, NH, TB], f16, tag="rot", bufs=2)
                nc.sync.dma_start(rotb[0:64, :, :], qb[64:128, :, :])
                nc.sync.dma_start(rotb[64:128, :, :], qb[0:64, :, :])
                t1 = wk_.tile([128, NH, TB], f16, tag="t12", bufs=4)
                t2 = wk_.tile([128, NH, TB], f16, tag="t12", bufs=4)
                nc.vector.tensor_tensor(t1[:], qb[:], bc2(cos_t[:, tbs]), OP.mult)
                nc.vector.tensor_tensor(t2[:], rotb[:], bc2(sin_t[:, tbs]), OP.mult)
                if is_q:
                    nc.vector.tensor_tensor(qT[:], t1[:], t2[:], OP.add)
                else:
                    nc.vector.tensor_tensor(ktab[:, :, tbs], t1[:], t2[:], OP.add)

            def emit_v(b, tb, xts, use_act=False):
                """use_act: during the cold start the DVE queue is saturated
                with RoPE work, so the first blocks' v-cache evacuations ride
                the then-idle ACT engine instead."""
                _, vt = kv_state[b]

                def xc(kc):
                    return xts[kc // 4][:, kc % 4, :]

                for tt in range(4):
                    vps = ps.tile([128, NH, TB], f32, tag="pj", bufs=1)
                    for kc in range(CK):
                        nc.tensor.matmul(
                            vps[:, 0, : NH * HD],
                            xc(kc)[:, tt * 128 : (tt + 1) * 128],
                            wv_t[:, kc, :],
                            start=(kc == 0),
                            stop=(kc == CK - 1),
                        )
                    if use_act:
                        nc.scalar.activation(
                            vt[:, tb * 4 + tt, :],
                            vps[:, 0, : NH * HD],
                            AF.Identity,
                        )
                    else:
                        nc.vector.tensor_copy(
                            vt[:, tb * 4 + tt, :], vps[:, 0, : NH * HD]
                        )

            def emit_proj(b, tb, xts):
                """q/k projections + RoPE (heads paired) and v projection."""
                if tb == 0:
                    alloc_kv(b)
                ktab, vt = kv_state[b]

                def xc(kc):
                    return xts[kc // 4][:, kc % 4, :]

                qT = wk_.tile([128, NH, TB], f16, tag="qT", bufs=2)
                for qk, b_t, is_q in ((0, bq_t, True), (1, bk_t, False)):
                    prj = ps.tile([128, NH, TB], f32, tag="pj", bufs=1)
                    for h in range(NH):
                        hsl = slice(h * HD, (h + 1) * HD)
                        for kc in range(CK):
                            nc.tensor.matmul(
                                prj[:, h, :],
                                wqk_t[:, kc, qk, hsl],
                                xc(kc),
                                start=(kc == 0),
                                stop=(kc == CK - 1),
                            )
                    emit_rope(b, tb, prj, b_t, is_q, qT)
                emit_v(b, tb, xts)
                return qT

            def emit_proj_first2(xts0, xts1):
                """Blocks (0,0)+(0,1) q/k projections interleaved per
                contraction chunk: the cold-start weight stream then needs
                ~300GB/s instead of ~450GB/s, so the PE is never DMA-starved
                while wqk/x stream in."""
                alloc_kv(0)
                qT0 = wk_.tile([128, NH, TB], f16, tag="qT", bufs=2, name="qT0")
                qT1 = wk_.tile([128, NH, TB], f16, tag="qT", bufs=2, name="qT1")
                prj_q0 = ps.tile([128, NH, TB], f32, tag="pj", bufs=1, name="prj_q0")
                prj_k0 = ps.tile([128, NH, TB], f32, tag="s", bufs=2, name="prj_k0")
                prj_q1 = ps.tile([128, NH, TB], f32, tag="s", bufs=2, name="prj_q1")
                prj_k1 = ps.tile([128, NH, TB], f32, tag="y", bufs=1, name="prj_k1")

                def xc0(kc):
                    return xts0[kc // 4][:, kc % 4, :]

                def xc1(kc):
                    return xts1[kc // 4][:, kc % 4, :]

                for kc in range(CK):
                    for qk, prjs in ((0, (prj_q0, prj_q1)), (1, (prj_k0, prj_k1))):
                        for h in range(NH):
                            hsl = slice(h * HD, (h + 1) * HD)
                            for prjt, xcf in zip(prjs, (xc0, xc1)):
                                nc.tensor.matmul(
                                    prjt[:, h, :],
                                    wqk_t[:, kc, qk, hsl],
                                    xcf(kc),
                                    start=(kc == 0),
                                    stop=(kc == CK - 1),
                                )
                # q0's rope first: its PSUM evacuation releases the only
                # "pj" slot, unblocking v(0,0)'s matmuls; k0's chain then
                # runs on ACT/DVE underneath them. Block-0 inputs before
                # block-1 so attention(0) starts while block-1 still projects.
                emit_rope(0, 0, prj_q0, bq_t, True, qT0)
                emit_rope(0, 0, prj_k0, bk_t, False, None)
                emit_v(0, 0, xts0, use_act=True)
                emit_rope(0, 1, prj_k1, bk_t, False, None)
                emit_rope(0, 1, prj_q1, bq_t, True, qT1)
                emit_v(0, 1, xts1, use_act=True)
                return qT0, qT1

            def emit_attn(b, tb, qT):
                """Causal softmax(qk^T)v for one query block, heads paired."""
                ktab, vt = kv_state[b]
                nkt = 4 * tb + 4
                y_ps = ps.tile([128, NH, TB], f32, tag="y", bufs=1)
                dacc = wk_.tile([128, NH, TB], f16, tag="dacc", bufs=2)
                for kt in range(nkt):
                    # causal trim: key tile kt only reaches queries
                    # >= q0 = 128*(kt-4*tb); skip the dead columns
                    o = kt - 4 * tb
                    q0 = 128 * o if o > 0 else 0
                    s_ps = ps.tile([128, NH, TB], f32, tag="s", bufs=2)
                    for h in range(NH):
                        nc.tensor.matmul(
                            s_ps[:, h, q0:],
                            ktab[:, h, kt * 128 : (kt + 1) * 128],
                            qT[:, h, q0:],
                            start=True,
                            stop=(o < 0),
                        )
                        if o >= 0:
                            # causal mask: add -1e4 above the diagonal of the
                            # boundary 128x128 strip (maskm.T @ I)
                            nc.tensor.matmul(
                                s_ps[:, h, q0 : q0 + 128],
                                maskm_t[:],
                                ident_t[:],
                                start=False,
                                stop=True,
                                skip_group_check=True,
                            )
                    pt = wk_.tile([128, NH, TB], f16, tag="pt", bufs=4)
                    nc.scalar.activation(
                        pt[:, :, q0:], s_ps[:, :, q0:], AF.Exp, scale=SCALE
                    )
                    if kt == 0:
                        nc.vector.tensor_copy(dacc[:], pt[:])
                    elif kt < nkt - 1:
                        nc.vector.tensor_tensor(
                            dacc[:, :, q0:], dacc[:, :, q0:], pt[:, :, q0:], OP.add
                        )
                    else:
                        # the last key tile (diagonal o=3 strip, queries 384:)
                        # is NOT folded into dacc: the denominator matmul over
                        # dacc can then issue before the last exp finishes,
                        # and the missing strip is added as a cheap N=128
                        # accumulate right after it
                        pt_last = pt
                    for h in range(NH):
                        nc.tensor.matmul(
                            y_ps[:, h, q0:],
                            vt[:, kt, h * HD : (h + 1) * HD],
                            pt[:, h, q0:],
                            start=(kt == 0),
                            stop=(kt == nkt - 1),
                            skip_group_check=True,
                        )
                den = ps.tile([128, NH, TB], f32, tag="s", bufs=2)
                for h in range(NH):
                    nc.tensor.matmul(
                        den[:, h, :], ones_t[:], dacc[:, h, :], start=True, stop=False
                    )
                for h in range(NH):
                    nc.tensor.matmul(
                        den[:, h, 384:],
                        ones_t[:],
                        pt_last[:, h, 384:],
                        start=False,
                        stop=True,
                        skip_group_check=True,
                    )
                rden = wk_.tile([128, NH, TB], f32, tag="rden", bufs=2)
                yT = wk_.tile([128, NH, TB], f16, tag="yT", bufs=2)
                for h in range(NH):
                    nc.vector.reciprocal_approx_fast(rden[:, h, :], den[:, h, :])
                    nc.vector.tensor_tensor(
                        yT[:, h, :], y_ps[:, h, :], rden[:, h, :], OP.mult
                    )
                return yT

            def emit_out(b, tb, yT, last=False):
                """Partial output projection for one block + store. For the
                final row of the last block, evacuation and stores are split
                per 512-column half so the kernel-tail critical path is one
                small copy + one small DMA instead of a paired one."""
                # after the final attention there are no more projections,
                # so the last block's out groups rotate through 4 PSUM slots
                # (s,s,pj,y) instead of 2 and split every evacuation across
                # ACT+DVE: the PE never idles long enough for the clock gate
                # to re-throttle during the kernel tail
                tags = ["s", "pj", "s", "y", "s", "pj", "s", "y"] if last else ["s"] * 8
                for tt in range(4):
                    r0 = tb * TB + tt * 128
                    for cp in range(2):
                        tg = tags[tt * 2 + cp]
                        o_ps = ps.tile(
                            [128, 2, TB], f32, tag=tg, bufs={"s": 2, "pj": 1, "y": 1}[tg]
                        )
                        for c2 in range(2):
                            csl = slice((cp * 2 + c2) * TB, (cp * 2 + c2 + 1) * TB)
                            for h in range(NH):
                                nc.tensor.matmul(
                                    o_ps[:, c2, :],
                                    yT[:, h, tt * 128 : (tt + 1) * 128],
                                    wo_t[:, h, csl],
                                    start=(h == 0),
                                    stop=(h == NH - 1),
                                )
                        ot = wk_.tile([128, 2, TB], f16, tag="ot", bufs=4)
                        if last:
                            if cp == 0:
                                nc.scalar.activation(
                                    ot[:, 0, :], o_ps[:, 0, :], AF.Identity
                                )
                                nc.vector.tensor_copy(ot[:, 1, :], o_ps[:, 1, :])
                            else:
                                nc.vector.tensor_copy(ot[:, 0, :], o_ps[:, 0, :])
                                nc.scalar.activation(
                                    ot[:, 1, :], o_ps[:, 1, :], AF.Identity
                                )
                        elif cp == 0:
                            nc.scalar.activation(ot[:], o_ps[:], AF.Identity)
                        else:
                            nc.vector.tensor_copy(ot[:], o_ps[:])
                        if last:
                            for c2 in range(2):
                                nc.sync.dma_start(
                                    d_out.ap()[
                                        b,
                                        r0 : r0 + 128,
                                        (cp * 2 + c2) * TB : (cp * 2 + c2 + 1) * TB,
                                    ],
                                    ot[:, c2, :],
                                )
                        else:
                            nc.sync.dma_start(
                                d_out.ap()[
                                    b, r0 : r0 + 128, cp * 1024 : (cp + 1) * 1024
                                ],
                                ot[:],
                            )

            # --- software-pipelined emission: attention(g) is emitted before
            # proj(g+1) (so the RoPE chain of g+1 hides under attention g),
            # and out(g) last (its PE work covers the proj->rope latency).
            # The first two blocks' projections are jointly emitted so the
            # cold-start weight stream stays under the HBM rate. ---
            blocks = [(b, tb) for b in range(B) for tb in range(NTB)]
            NB = len(blocks)
            # prologue: x(0,0), x(0,1) and wqk interleaved per j so the joint
            # first projection's three input streams arrive in lockstep; the
            # very first chunk of each stream ships alone so matmul 0 can
            # start after three small transfers
            xls = {0: [], 1: []}

            # PE warm-up: ~36 throwaway matmuls on a zeroed SBUF tile keep
            # the PE busy from ~6us (no DMA dependency), so the HAM clock
            # gate is already at 8/8 when the first projection matmul's
            # data lands (~10us). Emitted first = highest priority.
            garb = wk_.tile([128, 128], f16, tag="garb", bufs=1, name="garb")
            nc.gpsimd.memset(garb[:], 0.0)
            warm = ps.tile([128, 2, TB], f32, tag="s", bufs=2, name="warm")
            for i in range(36):
                nc.tensor.matmul(
                    warm[:, 0, 0:128],
                    garb[:],
                    garb[:],
                    start=(i == 0),
                    stop=(i == 35),
                )

            def alloc_x(b, tb, j):
                xt = xp.tile(
                    [128, 4, TB], f16, tag="xt", bufs=12, name=f"x{b}{tb}{j}"
                )
                return xt, d_xT.ap()[b, tb, j]

            for j in range(4):
                wsrc = d_wqk.ap()[j]
                if j == 0:
                    # first chunk of each stream ships alone: matmul 0 can
                    # start after three small transfers
                    xt0, src0 = alloc_x(0, 0, j)
                    xt1, src1 = alloc_x(0, 1, j)
                    nc.sync.dma_start(xt0[:, 0:1, :], src0[:, 0:1, :])
                    nc.sync.dma_start(wqk_t[:, 0:1, :, :], wsrc[:, 0:1, :, :])
                    nc.sync.dma_start(xt1[:, 0:1, :], src1[:, 0:1, :])
                    nc.sync.dma_start(xt0[:, 1:4, :], src0[:, 1:4, :])
                    nc.sync.dma_start(wqk_t[:, 1:4, :, :], wsrc[:, 1:4, :, :])
                    nc.sync.dma_start(xt1[:, 1:4, :], src1[:, 1:4, :])
                    xls[0].append(xt0)
                    xls[1].append(xt1)
                else:
                    xls[0].append(emit_x1(0, 0, j))
                    xls[1].append(emit_x1(0, 1, j))
                    nc.sync.dma_start(wqk_t[:, j * 4 : (j + 1) * 4, :, :], wsrc)
            load_constants()
            qTs = {}
            qTs[0], qTs[1] = emit_proj_first2(xls[0], xls[1])
            # out(g-1) is emitted AFTER attn(g)+proj(g+1): during attn(g)'s
            # exp stalls the scheduler consumes proj(g+1) first, so out work
            # is still available to fill the den->reciprocal->yT latency hole
            # at the end of attn(g)
            yTs = {}
            for g, (b, tb) in enumerate(blocks):
                if g + 2 < NB:
                    xls[g + 2] = emit_x(*blocks[g + 2])
                # early blocks: projections are emitted ahead of the (small)
                # attention so the cold-phase PE queue prefers them; from
                # g=3 attention leads so proj/out fill its exp stalls
                if g <= 2 and g + 1 < NB and (g + 1) not in qTs:
                    qTs[g + 1] = emit_proj(*blocks[g + 1], xls[g + 1])
                yTs[g] = emit_attn(b, tb, qTs.pop(g))
                if g + 1 < NB and (g + 1) not in qTs:
                    qTs[g + 1] = emit_proj(*blocks[g + 1], xls[g + 1])
                del xls[g],
                if g >= 1:
                    emit_out(*blocks[g - 1], yTs.pop(g - 1))
            emit_out(*blocks[NB - 1], yTs.pop(NB - 1), last=True)

    nc.compile()
    return nc



def _get_program():
    if "nc" not in _STATE:
        _STATE["nc"] = _build_program()
    return _STATE["nc"]


def _enable_trace_hooks():
    import types

    import antenv

    if not hasattr(antenv, "axon_hooks"):
        hooks_mod = types.ModuleType("antenv.axon_hooks")
        _hook = [None]
        hooks_mod.set_axon_ntff_profile_hook = lambda h: _hook.__setitem__(0, h)
        hooks_mod.get_axon_ntff_profile_hook = lambda: _hook[0]
        sys.modules["antenv.axon_hooks"] = hooks_mod
        antenv.axon_hooks = hooks_mod
        from trn_agent_boot.trn_boot import _ntff_profile_via_ctypes

        hooks_mod.set_axon_ntff_profile_hook(
            _ntff_profile_via_ctypes("/opt/axon/libaxon_pjrt.so")
        )
    bass_utils.upload_artifacts = lambda tmpdir: f"local://{tmpdir}"


def kernel(x, Wqkv, bqkv, Wo, bo):
    global LAST_RESULT
    x = np.asarray(x, dtype=np.float32)
    Wqkv = np.asarray(Wqkv, dtype=np.float32)
    bqkv = np.asarray(bqkv, dtype=np.float32)
    Wo = np.asarray(Wo, dtype=np.float32)
    bo = np.asarray(bo, dtype=np.float32)

    nc = _get_program()

    cosT, sinT = _rope_tables()
    sinT = sinT.copy()
    sinT[: HD // 2, :] *= -1.0  # rotation sign folded into the sin table
    cosT = np.ascontiguousarray(cosT.astype(np.float16))
    sinT = np.ascontiguousarray(sinT.astype(np.float16))
    onesm = np.ones((128, 128), dtype=np.float16)
    # additive causal mask M[j, i] = -1e4 where key j > query i (within the
    # diagonal 128x128 tile); applied on PE as maskm.T @ I, so pass M.T
    i_idx = np.arange(128)[None, :]
    j_idx = np.arange(128)[:, None]
    maskM = np.where(j_idx <= i_idx, np.float16(0), np.float16(-1e4))
    maskm = np.ascontiguousarray(maskM.T.astype(np.float16))
    ident = np.ascontiguousarray(np.eye(128, dtype=np.float16))
    xT = x.transpose(0, 2, 1).astype(np.float16)  # [B, C, T]
    # tile to [B, tb, j, p, c, t]: transfer (b,tb,j) is one contiguous 512KB
    xT = np.ascontiguousarray(
        xT.reshape(B, 4, 4, 128, NTB, TB).transpose(0, 4, 1, 3, 2, 5)
    )

    in_maps = []
    for c in range(NC_):
        rs = slice(c * NH * HD, (c + 1) * NH * HD)
        wq_c = Wqkv[0 * C :][rs.start : rs.stop, :].T.astype(np.float16)
        wk_c = Wqkv[1 * C :][rs.start : rs.stop, :].T.astype(np.float16)
        in_maps.append(
            {
                "xT": xT,
                "wqk": np.ascontiguousarray(
                    np.stack([wq_c, wk_c], axis=1)
                    .reshape(4, 4, 128, 2, NH * HD)
                    .transpose(0, 2, 1, 3, 4)
                ),
                "wv": np.ascontiguousarray(
                    Wqkv[2 * C :][rs.start : rs.stop, :]
                    .T.astype(np.float16)
                    .reshape(CK, 128, NH * HD)
                    .transpose(1, 0, 2)
                ),
                "wo": np.ascontiguousarray(Wo[:, rs].T.astype(np.float16)),
                "bq": np.ascontiguousarray(bqkv[0 * C :][rs].reshape(NH, HD).T),
                "bk": np.ascontiguousarray(bqkv[1 * C :][rs].reshape(NH, HD).T),
                "cosT": cosT,
                "sinT": sinT,
                "maskm": maskm,
                "ident": ident,
                "onesm": onesm,
            }
        )

    if TRACE:
        _enable_trace_hooks()
    res = bass_utils.run_bass_kernel_spmd(
        nc, in_maps, core_ids=list(range(NC_)), trace=TRACE
    )
    LAST_RESULT = res

    out = np.zeros((B, T, C), dtype=np.float64)
    for c in range(NC_):
        out += res.results[c]["out"].astype(np.float64)
    bv = bqkv[2 * C : 3 * C]
    out += (bo + Wo @ bv)[None, None, :]
    return out.astype(np.float32)



# revision 37
# speedup vs baseline: 1.0063x; 1.0063x over previous
"""Causal self-attention (B=2, T=2048, C=2048, H=16) on 8 TRN2 NeuronCores.

Sharding: tensor-parallel over heads (2 heads per core, both batches on every
core). Each core computes q/k/v projections for its 2 heads, RoPE, causal
softmax(qk^T)v, and a partial output projection against its slice of Wo's
columns. The host sums the 8 partial projections and adds the (linear) bias
terms.

v3 additions over v2:
  - x/wqk/wv ship pre-tiled in SBUF layout: every input transfer is a single
    contiguous block (4-16KB per-partition lines), roughly doubling the
    effective HBM rate during the DMA-bound cold start.
  - ~36 throwaway matmuls on a zeroed SBUF tile run during the initial DMA
    wait so the PE clock-gate (HAM) is already at 8/8 when real data lands.
  - RoPE tables are read through 0-stride broadcast APs over the head dim
    (no on-device duplication pass).
  - out(g-1) is emitted after attn(g)+proj(g+1), keeping out matmuls in
    reserve to fill the denominator->reciprocal->yT latency hole at the end
    of each attention block.
  - The softmax denominator is computed from the partial row-sum before the
    last exp finishes plus a cheap N=128 accumulate for the final strip.
  - The last block's out groups rotate through 4 PSUM slots (s,s,pj,y — the
    projection slots are dead by then) with ACT/DVE-split evacuation and
    per-half stores, so the kernel tail never idles the PE long enough to
    re-throttle the clock.

v2 layout strategy (all matmul operands fp16, PSUM accumulation fp32):
  - Host pre-casts x (transposed to xT [B, C, T]), weights, RoPE tables and
    masks to fp16: halves DMA bytes, removes the gpsimd casting-DMA path, and
    makes every matmul 1 cycle/row regardless of free-dim size.
  - The core's two heads are paired into single wide instructions wherever an
    ACT/DVE op would otherwise run per head: one [128, 2, 512] tile spans two
    PSUM banks (or 2KB of SBUF), so exp/mask/row-accumulate/copy pay their
    fixed per-instruction overhead once instead of twice.
  - PSUM budget (8 banks): "pj" proj pair (2) + "s" score/den/out pairs
    (2x2, double-buffered) + "y" attention accumulator pair (2). Separate
    tags keep next-block projections from queueing behind current-block
    output copies in the slot rotation.
  - Software-pipelined emission: attention(g) is emitted before proj(g+1)
    and out(g) last, so the static scheduler fills the exp round-trips of
    the attention inner loop with projection matmuls and covers the RoPE
    chain latency with output-projection work. The first two blocks'
    projections are interleaved per contraction chunk so the cold-start
    x/wqk stream stays under the HBM rate (the first chunk of each stream
    ships alone so matmul 0 issues after three small transfers).
  - q, k are produced transposed (qT/kT [head_dim, T]); scores are computed
    as S_T = kT_tile.T @ qT [keys, queries] so the softmax normalizer is a
    ones-matmul partition sum. y is produced transposed (yT = v_tile.T @ P_T)
    and feeds the output projection with no transposes anywhere.
  - exp via ACT (scale fused), fp16 out; causal masking is applied on the
    PE by accumulating a constant -1e4 upper-triangular matrix (maskm.T @ I)
    onto the diagonal-boundary 128-column strip of each score tile.
  - Row-sum accumulation (softmax denominator) runs on DVE in fp16 at 2x
    mode; the final partition-sum/broadcast is one ones-matmul per head.
"""

import sys

sys.path.insert(0, "/opt/trn_rl_repo")

import numpy as np

import concourse.bacc as bacc
import concourse.mybir as mybir
import concourse.tile as tile
from concourse import bass_utils
from concourse.bass import AP

B, T, C, H = 2, 2048, 2048, 16
HD = C // H  # 128
BASE = 10000.0
NC_ = 8  # cores
NH = H // NC_  # heads per core = 2
TB = 512  # T block
NTB = T // TB  # 4
CK = C // 128  # 16 contraction chunks
SCALE = 1.0 / float(np.sqrt(np.float32(HD)))

f32 = mybir.dt.float32
f16 = mybir.dt.float16
AF = mybir.ActivationFunctionType
OP = mybir.AluOpType

TRACE = False
LAST_RESULT = None

_STATE = {}


def _rope_tables():
    """cos/sin tables [HD, T] mirroring reference._rope_tables (f32 chain)."""
    try:
        import jax
        import jax.numpy as jnp

        cpu = jax.devices("cpu")[0]
        with jax.default_device(cpu):
            p = jnp.arange(HD // 2, dtype=jnp.float32)
            theta = jnp.power(BASE, -(2.0**p) / HD)
            pos = jnp.arange(1, T + 1, dtype=jnp.float32)[:, None]
            c = pos * theta
            ang = jnp.concatenate([c, c], axis=-1)  # [T, HD]
            cos = np.asarray(jnp.cos(ang)).T  # [HD, T]
            sin = np.asarray(jnp.sin(ang)).T
        return np.ascontiguousarray(cos), np.ascontiguousarray(sin)
    except Exception:
        p = np.arange(HD // 2, dtype=np.float32)
        theta = np.power(np.float32(BASE), (-(2.0**p) / HD).astype(np.float32))
        pos = np.arange(1, T + 1, dtype=np.float32)[:, None]
        c = (pos * theta).astype(np.float32)
        ang = np.concatenate([c, c], axis=-1)
        return (
            np.ascontiguousarray(np.cos(ang).T.astype(np.float32)),
            np.ascontiguousarray(np.sin(ang).T.astype(np.float32)),
        )


def _build_program():
    nc = bacc.Bacc("TRN2", target_bir_lowering=False, debug=False, num_devices=NC_)

    # x and wqk ship pre-tiled in SBUF layout so every transfer is fully
    # contiguous (4-16KB per-partition lines instead of 1KB strided lines:
    # ~2x the effective HBM rate during the cold start)
    d_xT = nc.dram_tensor("xT", (B, NTB, 4, 128, 4, TB), f16, kind="ExternalInput")
    d_wqk = nc.dram_tensor("wqk", (4, 128, 4, 2, NH * HD), f16, kind="ExternalInput")
    d_wv = nc.dram_tensor("wv", (128, CK, NH * HD), f16, kind="ExternalInput")
    d_wo = nc.dram_tensor("wo", (NH * HD, C), f16, kind="ExternalInput")
    d_bq = nc.dram_tensor("bq", (HD, NH), f32, kind="ExternalInput")
    d_bk = nc.dram_tensor("bk", (HD, NH), f32, kind="ExternalInput")
    d_cos = nc.dram_tensor("cosT", (HD, T), f16, kind="ExternalInput")
    d_sin = nc.dram_tensor("sinT", (HD, T), f16, kind="ExternalInput")
    d_maskm = nc.dram_tensor("maskm", (128, 128), f16, kind="ExternalInput")
    d_ident = nc.dram_tensor("ident", (128, 128), f16, kind="ExternalInput")
    d_ones = nc.dram_tensor("onesm", (128, 128), f16, kind="ExternalInput")
    d_out = nc.dram_tensor("out", (B, T, C), f16, kind="ExternalOutput")

    with tile.TileContext(nc) as tc:
        with (
            tc.tile_pool(name="w", bufs=1) as wp,
            tc.tile_pool(name="xp", bufs=1) as xp,
            tc.tile_pool(name="kv", bufs=1) as kvp,
            tc.tile_pool(name="work", bufs=1) as wk_,
            tc.tile_pool(name="ps", bufs=1, space="PSUM") as ps,
        ):
            # --- resident weights/constants ---
            # The first block's x tiles + wqk chunks are emitted first so they
            # win HWDGE/DMA priority; small constants ride the ACT HWDGE
            # queue; big late-use tables (cos/sin/wv/wo) are emitted in
            # first-use order behind them.
            wqk_t = wp.tile([128, CK, 2, NH * HD], f16, name="wqk_t")
            wv_t = wp.tile([128, CK, NH * HD], f16, name="wv_t")
            wo_t = wp.tile([128, NH, C], f16, name="wo_t")
            bq_t = wp.tile([128, NH], f32, name="bq_t")
            bk_t = wp.tile([128, NH], f32, name="bk_t")
            cos_t = wp.tile([128, T], f16, name="cos_t")
            sin_t = wp.tile([128, T], f16, name="sin_t")
            maskm_t = wp.tile([128, 128], f16, name="maskm_t")
            ident_t = wp.tile([128, 128], f16, name="ident_t")
            ones_t = wp.tile([128, 128], f16, name="ones_t")

            def load_constants():
                nc.sync.dma_start(bq_t[:], d_bq.ap()[:])
                nc.sync.dma_start(bk_t[:], d_bk.ap()[:])
                nc.sync.dma_start(maskm_t[:], d_maskm.ap()[:])
                nc.sync.dma_start(ident_t[:], d_ident.ap()[:])
                # big tables split in halves and ordered by first use so no
                # single transfer starves the first block's x/wqk stream
                nc.sync.dma_start(cos_t[:], d_cos.ap()[:])
                nc.sync.dma_start(sin_t[:], d_sin.ap()[:])
                nc.sync.dma_start(wv_t[:], d_wv.ap()[:])
                nc.sync.dma_start(ones_t[:], d_ones.ap()[:])
                for h in range(NH):
                    nc.sync.dma_start(
                        wo_t[:, h, :], d_wo.ap()[h * 128 : (h + 1) * 128, :]
                    )

            kv_state = {}

            def bc2(s):
                """[128, F] AP -> [128, NH, F] with a 0-stride head dim, so
                both heads' RoPE multiplies read the same table row."""
                return AP(s.tensor, s.offset, [list(s.ap[0]), [0, NH], list(s.ap[1])])

            def emit_x1(b, tb, j):
                """One x tile [128, 4, 512] (4 contraction chunks)."""
                xt = xp.tile([128, 4, TB], f16, tag="xt", bufs=12, name=f"x{b}{tb}{j}")
                nc.sync.dma_start(xt[:], d_xT.ap()[b, tb, j])
                return xt

            def emit_x(b, tb):
                """Stream one x block: 4 DMAs of [128, 4, 512]."""
                return [emit_x1(b, tb, j) for j in range(4)]

            def alloc_kv(b):
                ktab = kvp.tile([128, NH, T], f16, tag="kt", bufs=2, name=f"ktab_{b}")
                vt = kvp.tile(
                    [128, T // 128, NH * HD], f16, tag="v", bufs=2, name=f"vt_{b}"
                )
                kv_state[b] = (ktab, vt)

            def emit_rope(b, tb, prj, b_t, is_q, qT):
                """PSUM projection pair -> fp16 qT / ktab slice with RoPE.

                The two PSUM->SBUF bias copies are split across ACT and DVE so
                the projection PSUM slot frees after ~0.7us, not ~1.3us."""
                ktab, _ = kv_state[b]
                tbs = slice(tb * TB, (tb + 1) * TB)
                qb = wk_.tile([128, NH, TB], f16, tag="qb", bufs=2)
                nc.scalar.activation(
                    qb[:, 0, :], prj[:, 0, :], AF.Identity, bias=b_t[:, 0:1]
                )
                nc.vector.tensor_scalar_add(qb[:, 1, :], prj[:, 1, :], b_t[:, 1:2])
                rotb = wk_.tile([128, NH, TB], f16, tag="rot", bufs=2)
                nc.sync.dma_start(rotb[0:64, :, :], qb[64:128, :, :])
                nc.sync.dma_start(rotb[64:128, :, :], qb[0:64, :, :])
                t1 = wk_.tile([128, NH, TB], f16, tag="t12", bufs=4)
                t2 = wk_.tile([128, NH, TB], f16, tag="t12", bufs=4)
                nc.vector.tensor_tensor(t1[:], qb[:], bc2(cos_t[:, tbs]), OP.mult)
                nc.vector.tensor_tensor(t2[:], rotb[:], bc2(sin_t[:, tbs]), OP.mult)
                if is_q:
                    nc.vector.tensor_tensor(qT[:], t1[:], t2[:], OP.add)
                else:
                    nc.vector.tensor_tensor(ktab[:, :, tbs], t1[:], t2[:], OP.add)

            def emit_v(b, tb, xts, use_act=False):
                """use_act: during the cold start the DVE queue is saturated
                with RoPE work, so the first blocks' v-cache evacuations ride
                the then-idle ACT engine instead."""
                _, vt = kv_state[b]

                def xc(kc):
                    return xts[kc // 4][:, kc % 4, :]

                for tt in range(4):
                    vps = ps.tile([128, NH, TB], f32, tag="pj", bufs=1)
                    for kc in range(CK):
                        nc.tensor.matmul(
                            vps[:, 0, : NH * HD],
                            xc(kc)[:, tt * 128 : (tt + 1) * 128],
                            wv_t[:, kc, :],
                            start=(kc == 0),
                            stop=(kc == CK - 1),
                        )
                    if use_act:
                        nc.scalar.activation(
                            vt[:, tb * 4 + tt, :],
                            vps[:, 0, : NH * HD],
                            AF.Identity,
                        )
                    else:
                        nc.vector.tensor_copy(
                            vt[:, tb * 4 + tt, :], vps[:, 0, : NH * HD]
                        )

            def emit_proj(b, tb, xts):
                """q/k projections + RoPE (heads paired) and v projection."""
                if tb == 0:
                    alloc_kv(b)
                ktab, vt = kv_state[b]

                def xc(kc):
                    return xts[kc // 4][:, kc % 4, :]

                qT = wk_.tile([128, NH, TB], f16, tag="qT", bufs=2)
                for qk, b_t, is_q in ((0, bq_t, True), (1, bk_t, False)):
                    prj = ps.tile([128, NH, TB], f32, tag="pj", bufs=1)
                    for h in range(NH):
                        hsl = slice(h * HD, (h + 1) * HD)
                        for kc in range(CK):
                            nc.tensor.matmul(
                                prj[:, h, :],
                                wqk_t[:, kc, qk, hsl],
                                xc(kc),
                                start=(kc == 0),
                                stop=(kc == CK - 1),
                            )
                    emit_rope(b, tb, prj, b_t, is_q, qT)
                emit_v(b, tb, xts)
                return qT

            def emit_proj_first2(xts0, xts1):
                """Blocks (0,0)+(0,1) q/k projections interleaved per
                contraction chunk: the cold-start weight stream then needs
                ~300GB/s instead of ~450GB/s, so the PE is never DMA-starved
                while wqk/x stream in."""
                alloc_kv(0)
                qT0 = wk_.tile([128, NH, TB], f16, tag="qT", bufs=2, name="qT0")
                qT1 = wk_.tile([128, NH, TB], f16, tag="qT", bufs=2, name="qT1")
                prj_q0 = ps.tile([128, NH, TB], f32, tag="pj", bufs=1, name="prj_q0")
                prj_k0 = ps.tile([128, NH, TB], f32, tag="s", bufs=2, name="prj_k0")
                prj_q1 = ps.tile([128, NH, TB], f32, tag="s", bufs=2, name="prj_q1")
                prj_k1 = ps.tile([128, NH, TB], f32, tag="y", bufs=1, name="prj_k1")

                def xc0(kc):
                    return xts0[kc // 4][:, kc % 4, :]

                def xc1(kc):
                    return xts1[kc // 4][:, kc % 4, :]

                for kc in range(CK):
                    for qk, prjs in ((0, (prj_q0, prj_q1)), (1, (prj_k0, prj_k1))):
                        for h in range(NH):
                            hsl = slice(h * HD, (h + 1) * HD)
                            for prjt, xcf in zip(prjs, (xc0, xc1)):
                                nc.tensor.matmul(
                                    prjt[:, h, :],
                                    wqk_t[:, kc, qk, hsl],
                                    xcf(kc),
                                    start=(kc == 0),
                                    stop=(kc == CK - 1),
                                )
                # q0's rope first: its PSUM evacuation releases the only
                # "pj" slot, unblocking v(0,0)'s matmuls; k0's chain then
                # runs on ACT/DVE underneath them. Block-0 inputs before
                # block-1 so attention(0) starts while block-1 still projects.
                emit_rope(0, 0, prj_q0, bq_t, True, qT0)
                emit_rope(0, 0, prj_k0, bk_t, False, None)
                emit_v(0, 0, xts0, use_act=True)
                emit_rope(0, 1, prj_k1, bk_t, False, None)
                emit_rope(0, 1, prj_q1, bq_t, True, qT1)
                emit_v(0, 1, xts1, use_act=True)
                return qT0, qT1

            def emit_attn(b, tb, qT):
                """Causal softmax(qk^T)v for one query block, heads paired."""
                ktab, vt = kv_state[b]
                nkt = 4 * tb + 4
                y_ps = ps.tile([128, NH, TB], f32, tag="y", bufs=1)
                dacc = wk_.tile([128, NH, TB], f16, tag="dacc", bufs=2)
                for kt in range(nkt):
                    # causal trim: key tile kt only reaches queries
                    # >= q0 = 128*(kt-4*tb); skip the dead columns
                    o = kt - 4 * tb
                    q0 = 128 * o if o > 0 else 0
                    s_ps = ps.tile([128, NH, TB], f32, tag="s", bufs=2)
                    for h in range(NH):
                        nc.tensor.matmul(
                            s_ps[:, h, q0:],
                            ktab[:, h, kt * 128 : (kt + 1) * 128],
                            qT[:, h, q0:],
                            start=True,
                            stop=(o < 0),
                        )
                        if o >= 0:
                            # causal mask: add -1e4 above the diagonal of the
                            # boundary 128x128 strip (maskm.T @ I)
                            nc.tensor.matmul(
                                s_ps[:, h, q0 : q0 + 128],
                                maskm_t[:],
                                ident_t[:],
                                start=False,
                                stop=True,
                                skip_group_check=True,
                            )
                    pt = wk_.tile([128, NH, TB], f16, tag="pt", bufs=4)
                    nc.scalar.activation(
                        pt[:, :, q0:], s_ps[:, :, q0:], AF.Exp, scale=SCALE
                    )
                    if kt == 0:
                        nc.vector.tensor_copy(dacc[:], pt[:])
                    elif kt < nkt - 1:
                        nc.vector.tensor_tensor(
                            dacc[:, :, q0:], dacc[:, :, q0:], pt[:, :, q0:], OP.add
                        )
                    else:
                        # the last key tile (diagonal o=3 strip, queries 384:)
                        # is NOT folded into dacc: the denominator matmul over
                        # dacc can then issue before the last exp finishes,
                        # and the missing strip is added as a cheap N=128
                        # accumulate right after it
                        pt_last = pt
                    for h in range(NH):
                        nc.tensor.matmul(
                            y_ps[:, h, q0:],
                            vt[:, kt, h * HD : (h + 1) * HD],
                            pt[:, h, q0:],
                            start=(kt == 0),
                            stop=(kt == nkt - 1),
                            skip_group_check=True,
                        )
                den = ps.tile([128, NH, TB], f32, tag="s", bufs=2)
                for h in range(NH):
                    nc.tensor.matmul(
                        den[:, h, :], ones_t[:], dacc[:, h, :], start=True, stop=False
                    )
                for h in range(NH):
                    nc.tensor.matmul(
                        den[:, h, 384:],
                        ones_t[:],
                        pt_last[:, h, 384:],
                        start=False,
                        stop=True,
                        skip_group_check=True,
                    )
                rden = wk_.tile([128, NH, TB], f32, tag="rden", bufs=2)
                yT = wk_.tile([128, NH, TB], f16, tag="yT", bufs=2)
                for h in range(NH):
                    nc.vector.reciprocal_approx_fast(rden[:, h, :], den[:, h, :])
                    nc.vector.tensor_tensor(
                        yT[:, h, :], y_ps[:, h, :], rden[:, h, :], OP.mult
                    )
                return yT

            def emit_out(b, tb, yT, last=False):
                """Partial output projection for one block + store. For the
                final row of the last block, evacuation and stores are split
                per 512-column half so the kernel-tail critical path is one
                small copy + one small DMA instead of a paired one."""
                # after the final attention there are no more projections,
                # so the last block's out groups rotate through 4 PSUM slots
                # (s,s,pj,y) instead of 2 and split every evacuation across
                # ACT+DVE: the PE never idles long enough for the clock gate
                # to re-throttle during the kernel tail
                tags = ["s", "pj", "s", "y", "s", "pj", "s", "y"] if last else ["s"] * 8
                for tt in range(4):
                    r0 = tb * TB + tt * 128
                    for cp in range(2):
                        tg = tags[tt * 2 + cp]
                        o_ps = ps.tile(
                            [128, 2, TB], f32, tag=tg, bufs={"s": 2, "pj": 1, "y": 1}[tg]
                        )
                        for c2 in range(2):
                            csl = slice((cp * 2 + c2) * TB, (cp * 2 + c2 + 1) * TB)
                            for h in range(NH):
                                nc.tensor.matmul(
                                    o_ps[:, c2, :],
                                    yT[:, h, tt * 128 : (tt + 1) * 128],
                                    wo_t[:, h, csl],
                                    start=(h == 0),
                                    stop=(h == NH - 1),
                                )
                        ot = wk_.tile([128, 2, TB], f16, tag="ot", bufs=4)
                        if last:
                            if cp == 0:
                                nc.scalar.activation(
                                    ot[:, 0, :], o_ps[:, 0, :], AF.Identity
                                )
                                nc.vector.tensor_copy(ot[:, 1, :], o_ps[:, 1, :])
                            else:
                                nc.vector.tensor_copy(ot[:, 0, :], o_ps[:, 0, :])
                                nc.scalar.activation(
                                    ot[:, 1, :], o_ps[:, 1, :], AF.Identity
                                )
                        elif cp == 0:
                            nc.scalar.activation(ot[:], o_ps[:], AF.Identity)
                        else:
                            nc.vector.tensor_copy(ot[:], o_ps[:])
                        if last:
                            for c2 in range(2):
                                nc.sync.dma_start(
                                    d_out.ap()[
                                        b,
                                        r0 : r0 + 128,
                                        (cp * 2 + c2) * TB : (cp * 2 + c2 + 1) * TB,
                                    ],
                                    ot[:, c2, :],
                                )
                        else:
                            nc.sync.dma_start(
                                d_out.ap()[
                                    b, r0 : r0 + 128, cp * 1024 : (cp + 1) * 1024
                                ],
                                ot[:],
                            )

            # --- software-pipelined emission: attention(g) is emitted before
            # proj(g+1) (so the RoPE chain of g+1 hides under attention g),
            # and out(g) last (its PE work covers the proj->rope latency).
            # The first two blocks' projections are jointly emitted so the
            # cold-start weight stream stays under the HBM rate. ---
            blocks = [(b, tb) for b in range(B) for tb in range(NTB)]
            NB = len(blocks)
            # prologue: x(0,0), x(0,1) and wqk interleaved per j so the joint
            # first projection's three input streams arrive in lockstep; the
            # very first chunk of each stream ships alone so matmul 0 can
            # start after three small transfers
            xls = {0: [], 1: []}

            # PE warm-up: ~36 throwaway matmuls on a zeroed SBUF tile keep
            # the PE busy from ~6us (no DMA dependency), so the HAM clock
            # gate is already at 8/8 when the first projection matmul's
            # data lands (~10us). Emitted first = highest priority.
            garb = wk_.tile([128, 128], f16, tag="garb", bufs=1, name="garb")
            nc.gpsimd.memset(garb[:], 0.0)
            warm = ps.tile([128, 2, TB], f32, tag="s", bufs=2, name="warm")
            for i in range(36):
                nc.tensor.matmul(
                    warm[:, 0, 0:128],
                    garb[:],
                    garb[:],
                    start=(i == 0),
                    stop=(i == 35),
                )

            def alloc_x(b, tb, j):
                xt = xp.tile(
                    [128, 4, TB], f16, tag="xt", bufs=12, name=f"x{b}{tb}{j}"
                )
                return xt, d_xT.ap()[b, tb, j]

            for j in range(4):
                wsrc = d_wqk.ap()[j]
                if j == 0:
                    # first chunk of each stream ships alone: matmul 0 can
                    # start after three small transfers
                    xt0, src0 = alloc_x(0, 0, j)
                    xt1, src1 = alloc_x(0, 1, j)
                    nc.sync.dma_start(xt0[:, 0:1, :], src0[:, 0:1, :])
                    nc.sync.dma_start(wqk_t[:, 0:1, :, :], wsrc[:, 0:1, :, :])
                    nc.sync.dma_start(xt1[:, 0:1, :], src1[:, 0:1, :])
                    nc.sync.dma_start(xt0[:, 1:4, :], src0[:, 1:4, :])
                    nc.sync.dma_start(wqk_t[:, 1:4, :, :], wsrc[:, 1:4, :, :])
                    nc.sync.dma_start(xt1[:, 1:4, :], src1[:, 1:4, :])
                    xls[0].append(xt0)
                    xls[1].append(xt1)
                else:
                    xls[0].append(emit_x1(0, 0, j))
                    xls[1].append(emit_x1(0, 1, j))
                    nc.sync.dma_start(wqk_t[:, j * 4 : (j + 1) * 4, :, :], wsrc)
            load_constants()
            qTs = {}
            qTs[0], qTs[1] = emit_proj_first2(xls[0], xls[1])
            # out(g-1) is emitted AFTER attn(g)+proj(g+1): during attn(g)'s
            # exp stalls the scheduler consumes proj(g+1) first, so out work
            # is still available to fill the den->reciprocal->yT latency hole
            # at the end of attn(g)
            yTs = {}
            for g, (b, tb) in enumerate(blocks):
                if g + 2 < NB:
                    xls[g + 2] = emit_x(*blocks[g + 2])
                yTs[g] = emit_attn(b, tb, qTs.pop(g))
                if g + 1 < NB and (g + 1) not in qTs:
                    qTs[g + 1] = emit_proj(*blocks[g + 1], xls[g + 1])
                del xls[g],
                if g >= 1:
                    emit_out(*blocks[g - 1], yTs.pop(g - 1))
            emit_out(*blocks[NB - 1], yTs.pop(NB - 1), last=True)

    nc.compile()
    return nc



def _get_program():
    if "nc" not in _STATE:
        _STATE["nc"] = _build_program()
    return _STATE["nc"]


def _enable_trace_hooks():
    import types

    import antenv

    if not hasattr(antenv, "axon_hooks"):
        hooks_mod = types.ModuleType("antenv.axon_hooks")
        _hook = [None]
        hooks_mod.set_axon_ntff_profile_hook = lambda h: _hook.__setitem__(0, h)
        hooks_mod.get_axon_ntff_profile_hook = lambda: _hook[0]
        sys.modules["antenv.axon_hooks"] = hooks_mod
        antenv.axon_hooks = hooks_mod
        from trn_agent_boot.trn_boot import _ntff_profile_via_ctypes

        hooks_mod.set_axon_ntff_profile_hook(
            _ntff_profile_via_ctypes("/opt/axon/libaxon_pjrt.so")
        )
    bass_utils.upload_artifacts = lambda tmpdir: f"local://{tmpdir}"


def kernel(x, Wqkv, bqkv, Wo, bo):
    global LAST_RESULT
    x = np.asarray(x, dtype=np.float32)
    Wqkv = np.asarray(Wqkv, dtype=np.float32)
    bqkv = np.asarray(bqkv, dtype=np.float32)
    Wo = np.asarray(Wo, dtype=np.float32)
    bo = np.asarray(bo, dtype=np.float32)

    nc = _get_program()

    cosT, sinT = _rope_tables()
    sinT = sinT.copy()
    sinT[: HD // 2, :] *= -1.0  # rotation sign folded into the sin table
    cosT = np.ascontiguousarray(cosT.astype(np.float16))
    sinT = np.ascontiguousarray(sinT.astype(np.float16))
    onesm = np.ones((128, 128), dtype=np.float16)
    # additive causal mask M[j, i] = -1e4 where key j > query i (within the
    # diagonal 128x128 tile); applied on PE as maskm.T @ I, so pass M.T
    i_idx = np.arange(128)[None, :]
    j_idx = np.arange(128)[:, None]
    maskM = np.where(j_idx <= i_idx, np.float16(0), np.float16(-1e4))
    maskm = np.ascontiguousarray(maskM.T.astype(np.float16))
    ident = np.ascontiguousarray(np.eye(128, dtype=np.float16))
    xT = x.transpose(0, 2, 1).astype(np.float16)  # [B, C, T]
    # tile to [B, tb, j, p, c, t]: transfer (b,tb,j) is one contiguous 512KB
    xT = np.ascontiguousarray(
        xT.reshape(B, 4, 4, 128, NTB, TB).transpose(0, 4, 1, 3, 2, 5)
    )

    in_maps = []
    for c in range(NC_):
        rs = slice(c * NH * HD, (c + 1) * NH * HD)
        wq_c = Wqkv[0 * C :][rs.start : rs.stop, :].T.astype(np.float16)
        wk_c = Wqkv[1 * C :][rs.start : rs.stop, :].T.astype(np.float16)
        in_maps.append(
            {
                "xT": xT,
                "wqk": np.ascontiguousarray(
                    np.stack([wq_c, wk_c], axis=1)
                    .reshape(4, 4, 128, 2, NH * HD)
                    .transpose(0, 2, 1, 3, 4)
                ),
                "wv": np.ascontiguousarray(
                    Wqkv[2 * C :][rs.start : rs.stop, :]
                    .T.astype(np.float16)
                    .reshape(CK, 128, NH * HD)
                    .transpose(1, 0, 2)
                ),
                "wo": np.ascontiguousarray(Wo[:, rs].T.astype(np.float16)),
                "bq": np.ascontiguousarray(bqkv[0 * C :][rs].reshape(NH, HD).T),
                "bk": np.ascontiguousarray(bqkv[1 * C :][rs].reshape(NH, HD).T),
                "cosT": cosT,
                "sinT": sinT,
                "maskm": maskm,
                "ident": ident,
                "onesm": onesm,
            }
        )

    if TRACE:
        _enable_trace_hooks()
    res = bass_utils.run_bass_kernel_spmd(
        nc, in_maps, core_ids=list(range(NC_)), trace=TRACE
    )
    LAST_RESULT = res

    out = np.zeros((B, T, C), dtype=np.float64)
    for c in range(NC_):
        out += res.results[c]["out"].astype(np.float64)
    bv = bqkv[2 * C : 3 * C]
    out += (bo + Wo @ bv)[None, None, :]
    return out.astype(np.float32)

